# revision 2
# baseline (speedup 1.0000x reference)
"""DistanceSVM forward on 8 TRN2 NeuronCores.

out[n] = max_avg_distance - sum_c w_c * ||x_n - center_c||,
w = |coefs| / sum(|coefs|)   (unnormalized if the sum is 0).

Strategy (data-parallel over N, centers/coefs replicated):
  - Fold the whole distance computation into one augmented GEMM:
        w_c^2 * d2[n,c] = [x_n, x2_n, 1] . [-2*u_c*center_c ; u_c ; u_c*c2_c]
    with u_c = w_c^2 >= 0, so  w_c * d[n,c] = sqrt(w_c^2 * d2[n,c])
    (d2 >= ~24 for this data, no relu needed).
  - TensorE (float32r) computes the augmented GEMM into PSUM.
  - ScalarE applies Sqrt; the weighted row-sum over c comes either from
    ScalarE's fused accumulator or from a VectorE reduce (the 1024-wide
    reduction is split ~6/7 onto DVE to keep ACT, the bottleneck engine,
    on pure sqrt work).
  - Host pre/post: builds the transposed augmented operands (layout
    choice only, O(N*D) numpy) and reassembles the sharded output.
"""

import numpy as np

import concourse.bacc as bacc
import concourse.bass as bass
import concourse.mybir as mybir
import concourse.tile as tile
from concourse.bass_utils import run_bass_kernel_spmd

N_CORES = 8
N, C, D = 131072, 1024, 64
NS = N // N_CORES            # rows per core
P = 128                      # partitions
TILES = NS // P              # n-tiles per core
K = D + 2                    # augmented contraction dim
NCHUNK = 8                   # input DMA chunks
DVE_MOD = 7                  # t % DVE_MOD == 0 -> ACT accumulates, else DVE reduces

_nc_cache = None


def _build_nc():
    f32 = mybir.dt.float32
    f32r = mybir.dt.float32r
    nc = bacc.Bacc("TRN2", target_bir_lowering=False)
    xaT = nc.dram_tensor("xaT", [K, NS], f32r, kind="ExternalInput")
    cw = nc.dram_tensor("cw", [K, C], f32r, kind="ExternalInput")
    mad = nc.dram_tensor("mad", [1], f32, kind="ExternalInput")
    out = nc.dram_tensor("out", [P, TILES], f32, kind="ExternalOutput")

    with tile.TileContext(nc) as tc:
        with tc.tile_pool(name="xp", bufs=1) as xp, \
             tc.tile_pool(name="singles", bufs=1) as singles, \
             tc.tile_pool(name="acc", bufs=1) as accp, \
             tc.tile_pool(name="ps", bufs=3, space="PSUM") as psp:
            cen = singles.tile([K, C], f32r, tag="cen")
            nc.sync.dma_start(out=cen, in_=cw[:, :])
            mad_sb = singles.tile([P, 1], f32, tag="mad")
            nc.sync.dma_start(out=mad_sb, in_=mad[:].to_broadcast((P, 1)))

            wa = accp.tile([P, TILES], f32, tag="wa")   # ACT-accumulated cols
            wd = accp.tile([P, TILES], f32, tag="wd")   # DVE-reduced cols
            nc.vector.memset(wa, 0.0)
            nc.vector.memset(wd, 0.0)

            xs = []
            csz = NS // NCHUNK
            for kk in range(NCHUNK):
                xt = xp.tile([K, csz], f32r, tag=f"x{kk}")
                nc.sync.dma_start(out=xt, in_=xaT[:, kk * csz:(kk + 1) * csz])
                xs.append(xt)

            per_chunk = TILES // NCHUNK
            for t in range(TILES):
                xt = xs[t // per_chunk]
                off = (t % per_chunk) * P
                lhsT = xt[:, off:off + P]
                ps = psp.tile([P, C], f32, tag="ps")
                nc.tensor.matmul(ps[:, 0:512], lhsT=lhsT, rhs=cen[:, 0:512],
                                 start=True, stop=True)
                nc.tensor.matmul(ps[:, 512:1024], lhsT=lhsT, rhs=cen[:, 512:1024],
                                 start=True, stop=True)
                if t % DVE_MOD == 0:
                    nc.scalar.activation(ps, ps, mybir.ActivationFunctionType.Sqrt,
                                         accum_out=wa[:, t:t + 1])
                else:
                    nc.scalar.sqrt(ps, ps)
                    nc.vector.tensor_reduce(out=wd[:, t:t + 1], in_=ps,
                                            op=mybir.AluOpType.add,
                                            axis=mybir.AxisListType.X)

            out_sb = accp.tile([P, TILES], f32, tag="os")
            nc.vector.tensor_tensor(out=wa, in0=wa, in1=wd,
                                    op=mybir.AluOpType.add)
            nc.vector.tensor_scalar(out=out_sb, in0=wa,
                                    scalar1=-1.0, scalar2=mad_sb,
                                    op0=mybir.AluOpType.mult,
                                    op1=mybir.AluOpType.add)
            nc.sync.dma_start(out=out[:, :], in_=out_sb)
    nc.finalize()
    return nc


def _get_nc():
    global _nc_cache
    if _nc_cache is None:
        _nc_cache = _build_nc()
    return _nc_cache


def build_in_maps(inputs, centers, coefs, max_avg_distance):
    x = np.ascontiguousarray(np.asarray(inputs, dtype=np.float32).reshape(N, D))
    cen = np.asarray(centers, dtype=np.float32)
    co = np.asarray(coefs, dtype=np.float32)
    mad = np.asarray(max_avg_distance, dtype=np.float32).reshape(1)

    w = np.abs(co)
    s = np.float32(w.sum(dtype=np.float32))
    if s != 0.0:
        w = (w / s).astype(np.float32)
    u = (w.astype(np.float64) ** 2)
    c2 = (cen.astype(np.float64) ** 2).sum(axis=1)

    cw = np.empty((K, C), dtype=np.float32)
    cw[:D] = (-2.0 * u[:, None] * cen.astype(np.float64)).T.astype(np.float32)
    cw[D] = u.astype(np.float32)
    cw[D + 1] = (u * c2).astype(np.float32)

    in_maps = []
    for g in range(N_CORES):
        xg = x[g * NS:(g + 1) * NS]
        xaT = np.empty((K, NS), dtype=np.float32)
        xaT[:D] = xg.T
        xaT[D] = (xg.astype(np.float64) ** 2).sum(axis=1).astype(np.float32)
        xaT[D + 1] = 1.0
        in_maps.append({"xaT": xaT, "cw": cw, "mad": mad})
    return in_maps


def kernel(inputs, centers, coefs, max_avg_distance):
    in_maps = build_in_maps(inputs, centers, coefs, max_avg_distance)
    res = run_bass_kernel_spmd(_get_nc(), in_maps, core_ids=list(range(N_CORES)))
    full = np.concatenate(
        [np.asarray(res.results[g]["out"]).T.reshape(-1) for g in range(N_CORES)]
    )
    return full.astype(np.float32)


# revision 6
# speedup vs baseline: 1.0371x; 1.0371x over previous
"""DistanceSVM forward on 8 TRN2 NeuronCores.

out[n] = max_avg_distance - sum_c w_c * ||x_n - center_c||,
w = |coefs| / sum(|coefs|)   (unnormalized if the sum is 0).

Strategy (data-parallel over N, centers/coefs replicated, per spec hint):
  - Fold the whole distance computation into one augmented GEMM:
        2^S * w_c^2 * d2[n,c] =
            [x_n, x2hi_n, x2lo_n, 1] . [-2*u_c*center_c ; u_c ; u_c ; u_c*c2_c]
    with u_c = 2^S * w_c^2 >= 0 (S rescales u into fp16-friendly range),
    so  w_c * d[n,c] = sqrt(2^-S * psum).  d2 >= ~24 for randn data in
    64-d, so no relu is needed before sqrt.  x2 is carried as an fp16
    hi/lo pair to keep the large self-term at ~fp32 accuracy.
  - TensorE (fp16 operands, fp32 PSUM accumulate, 1 cycle/row) computes
    the augmented GEMM: 4 x [128, 512] matmuls per [128, 2048] PSUM group
    (two 128-row n-tiles per group).
  - ScalarE applies Sqrt (with the free 2^-S prescale) in one [128, 2048]
    instruction per group, in place in PSUM.
  - VectorE folds each n-tile's two 512-wide halves with a fused
    tensor_tensor_reduce (add + row-sum) -> weighted average per row.
  - Final (128, TILES) epilogue: out = mad - wavg, then one contiguous DMA.
  - Host pre/post (numpy, O(N*D)): builds the transposed augmented fp16
    operands, reassembles the sharded output.
"""

import numpy as np

import concourse.bacc as bacc
import concourse.bass as bass
import concourse.mybir as mybir
import concourse.tile as tile
from concourse.bass_utils import run_bass_kernel_spmd

N_CORES = 8
N, C, D = 131072, 1024, 64
NS = N // N_CORES            # rows per core
P = 128                      # partitions
TILES = NS // P              # n-tiles per core (128)
K = D + 3                    # x, x2_hi, x2_lo, ones
S = 22                       # global exponent scale on u = w^2
NCHUNK = 16                  # input DMA chunks

_nc_cache = None


def _build_nc():
    f32 = mybir.dt.float32
    f16 = mybir.dt.float16
    nc = bacc.Bacc("TRN2", target_bir_lowering=False)
    xaT = nc.dram_tensor("xaT", [K, NS], f16, kind="ExternalInput")
    cw = nc.dram_tensor("cw", [K, C], f16, kind="ExternalInput")
    mad = nc.dram_tensor("mad", [1], f32, kind="ExternalInput")
    out = nc.dram_tensor("out", [P, TILES], f32, kind="ExternalOutput")

    with tile.TileContext(nc) as tc:
        with tc.tile_pool(name="xp", bufs=1) as xp, \
             tc.tile_pool(name="singles", bufs=1) as singles, \
             tc.tile_pool(name="acc", bufs=1) as accp, \
             tc.tile_pool(name="sq", bufs=2) as sqp, \
             tc.tile_pool(name="ps", bufs=2, space="PSUM") as psp:
            cen = singles.tile([K, C], f16, tag="cen")
            nc.sync.dma_start(out=cen, in_=cw[:, :])
            mad_sb = singles.tile([P, 1], f32, tag="mad")
            nc.sync.dma_start(out=mad_sb, in_=mad[:].to_broadcast((P, 1)))

            wa = accp.tile([P, TILES], f32, tag="wa")
            wd = accp.tile([P, TILES], f32, tag="wd")
            nc.vector.memset(wa, 0.0)
            nc.vector.memset(wd, 0.0)

            xs = []
            csz = NS // NCHUNK
            for kk in range(NCHUNK):
                xt = xp.tile([K, csz], f16, tag=f"x{kk}")
                nc.sync.dma_start(out=xt, in_=xaT[:, kk * csz:(kk + 1) * csz])
                xs.append(xt)

            tiles_per_chunk = csz // P
            add = mybir.AluOpType.add
            sqrt_fn = mybir.ActivationFunctionType.Sqrt
            inv_scale = float(2.0 ** (-S))
            for g in range(TILES // 2):
                ps = psp.tile([P, 2048], f32, tag="ps")
                for h in range(2):
                    t = 2 * g + h
                    xt = xs[t // tiles_per_chunk]
                    off = (t % tiles_per_chunk) * P
                    lhsT = xt[:, off:off + P]
                    base = h * 1024
                    nc.tensor.matmul(ps[:, base:base + 512], lhsT=lhsT,
                                     rhs=cen[:, 0:512], start=True, stop=True)
                    nc.tensor.matmul(ps[:, base + 512:base + 1024], lhsT=lhsT,
                                     rhs=cen[:, 512:1024], start=True, stop=True)
                if g % 9 == 8:
                    # ACT-group: fused sqrt + per-row accumulate, one n-tile
                    # per instruction (accumulator spans the whole free dim).
                    for h in range(2):
                        t = 2 * g + h
                        base = h * 1024
                        nc.scalar.activation(ps[:, base:base + 1024],
                                             ps[:, base:base + 1024],
                                             sqrt_fn, scale=inv_scale,
                                             accum_out=wa[:, t:t + 1])
                else:
                    # DVE-group: one wide sqrt on ACT, row-sums on DVE.
                    sq = sqp.tile([P, 2, 1024], f32, tag="sq")
                    nc.scalar.activation(sq, ps, sqrt_fn, scale=inv_scale)
                    nc.vector.tensor_reduce(out=wd[:, 2 * g:2 * g + 2], in_=sq,
                                            op=add, axis=mybir.AxisListType.X)

            out_sb = accp.tile([P, TILES], f32, tag="os")
            nc.vector.tensor_tensor(out=wd, in0=wd, in1=wa, op=add)
            nc.vector.tensor_scalar(out=out_sb, in0=wd,
                                    scalar1=-1.0, scalar2=mad_sb,
                                    op0=mybir.AluOpType.mult,
                                    op1=mybir.AluOpType.add)
            nc.sync.dma_start(out=out[:, :], in_=out_sb)
    nc.finalize()
    return nc


def _get_nc():
    global _nc_cache
    if _nc_cache is None:
        _nc_cache = _build_nc()
    return _nc_cache


def build_in_maps(inputs, centers, coefs, max_avg_distance):
    x = np.ascontiguousarray(np.asarray(inputs, dtype=np.float32).reshape(N, D))
    cen = np.asarray(centers, dtype=np.float32)
    co = np.asarray(coefs, dtype=np.float32)
    mad = np.asarray(max_avg_distance, dtype=np.float32).reshape(1)

    w = np.abs(co)
    s = np.float32(w.sum(dtype=np.float32))
    if s != 0.0:
        w = (w / s).astype(np.float32)
    u = (w.astype(np.float64) ** 2) * (2.0 ** S)
    c2 = (cen.astype(np.float64) ** 2).sum(axis=1)

    cw = np.empty((K, C), dtype=np.float16)
    cw[:D] = (-2.0 * u[:, None] * cen.astype(np.float64)).T.astype(np.float16)
    cw[D] = u.astype(np.float16)
    cw[D + 1] = cw[D]
    cw[D + 2] = (u * c2).astype(np.float16)

    in_maps = []
    for g in range(N_CORES):
        xg = x[g * NS:(g + 1) * NS]
        x2 = (xg.astype(np.float64) ** 2).sum(axis=1)
        x2_hi = x2.astype(np.float16)
        x2_lo = (x2 - x2_hi.astype(np.float64)).astype(np.float16)
        xaT = np.empty((K, NS), dtype=np.float16)
        xaT[:D] = xg.T.astype(np.float16)
        xaT[D] = x2_hi
        xaT[D + 1] = x2_lo
        xaT[D + 2] = 1.0
        in_maps.append({"xaT": xaT, "cw": cw, "mad": mad})
    return in_maps


def kernel(inputs, centers, coefs, max_avg_distance):
    in_maps = build_in_maps(inputs, centers, coefs, max_avg_distance)
    res = run_bass_kernel_spmd(_get_nc(), in_maps, core_ids=list(range(N_CORES)))
    full = np.concatenate(
        [np.asarray(res.results[g]["out"]).T.reshape(-1) for g in range(N_CORES)]
    )
    return full.astype(np.float32)


# revision 8
# speedup vs baseline: 1.1886x; 1.1461x over previous
"""DistanceSVM forward on 8 TRN2 NeuronCores.

out[n] = max_avg_distance - sum_c w_c * ||x_n - center_c||,
w = |coefs| / sum(|coefs|)   (unnormalized if the sum is 0).

Strategy (data-parallel over N, centers/coefs replicated, per spec hint):
  - Fold the whole distance computation into one augmented GEMM:
        2^S * w_c^2 * d2[n,c] =
            [x_n, x2hi_n, x2lo_n, 1] . [-2*u_c*center_c ; u_c ; u_c ; u_c*c2_c]
    with u_c = 2^S * w_c^2 >= 0 (S rescales u into fp16-friendly range),
    so  w_c * d[n,c] = sqrt(2^-S * psum).  d2 >= ~24 for randn data in
    64-d, so no relu is needed before sqrt.  x2 is carried as an fp16
    hi/lo pair to keep the large self-term at ~fp32 accuracy.
  - TensorE (fp16 operands, fp32 PSUM accumulate, 1 cycle/row) computes
    the augmented GEMM: 4 x [128, 512] matmuls per [128, 2048] PSUM group
    (two 128-row n-tiles per group).
  - ScalarE applies Sqrt (with the free 2^-S prescale) in one [128, 2048]
    instruction per group, in place in PSUM.
  - VectorE folds each n-tile's two 512-wide halves with a fused
    tensor_tensor_reduce (add + row-sum) -> weighted average per row.
  - Final (128, TILES) epilogue: out = mad - wavg, then one contiguous DMA.
  - Host pre/post (numpy, O(N*D)): builds the transposed augmented fp16
    operands, reassembles the sharded output.
"""

import numpy as np

import concourse.bacc as bacc
import concourse.bass as bass
import concourse.mybir as mybir
import concourse.tile as tile
from concourse.bass_utils import run_bass_kernel_spmd

N_CORES = 8
N, C, D = 131072, 1024, 64
NS = N // N_CORES            # rows per core
P = 128                      # partitions
TILES = NS // P              # n-tiles per core (128)
K = D + 3                    # x, x2_hi, x2_lo, ones
S = 22                       # global exponent scale on u = w^2
NCHUNK = 16                  # input DMA chunks

_nc_cache = None


def _build_nc():
    f32 = mybir.dt.float32
    f16 = mybir.dt.float16
    nc = bacc.Bacc("TRN2", target_bir_lowering=False)
    xaT = nc.dram_tensor("xaT", [K, NS], f16, kind="ExternalInput")
    cw = nc.dram_tensor("cw", [K, C], f16, kind="ExternalInput")
    mad = nc.dram_tensor("mad", [1], f32, kind="ExternalInput")
    out = nc.dram_tensor("out", [P, TILES], f32, kind="ExternalOutput")

    with tile.TileContext(nc) as tc:
        with tc.tile_pool(name="xp", bufs=1) as xp, \
             tc.tile_pool(name="singles", bufs=1) as singles, \
             tc.tile_pool(name="acc", bufs=1) as accp, \
             tc.tile_pool(name="sq", bufs=2) as sqp, \
             tc.tile_pool(name="ps", bufs=2, space="PSUM") as psp:
            cen = singles.tile([K, C], f16, tag="cen")
            nc.sync.dma_start(out=cen, in_=cw[:, :])
            mad_sb = singles.tile([P, 1], f32, tag="mad")
            nc.sync.dma_start(out=mad_sb, in_=mad[:].to_broadcast((P, 1)))

            wd = accp.tile([P, TILES], f32, tag="wd")

            xs = []
            csz = NS // NCHUNK
            for kk in range(NCHUNK):
                xt = xp.tile([K, csz], f16, tag=f"x{kk}")
                nc.sync.dma_start(out=xt, in_=xaT[:, kk * csz:(kk + 1) * csz])
                xs.append(xt)

            tiles_per_chunk = csz // P
            add = mybir.AluOpType.add
            sqrt_fn = mybir.ActivationFunctionType.Sqrt
            inv_scale = float(2.0 ** (-S))
            for g in range(TILES // 2):
                ps = psp.tile([P, 2048], f32, tag="ps")
                for h in range(2):
                    t = 2 * g + h
                    xt = xs[t // tiles_per_chunk]
                    off = (t % tiles_per_chunk) * P
                    lhsT = xt[:, off:off + P]
                    base = h * 1024
                    nc.tensor.matmul(ps[:, base:base + 512], lhsT=lhsT,
                                     rhs=cen[:, 0:512], start=True, stop=True)
                    nc.tensor.matmul(ps[:, base + 512:base + 1024], lhsT=lhsT,
                                     rhs=cen[:, 512:1024], start=True, stop=True)
                # One wide sqrt on ACT; per-tile halves-fold + row-sum on DVE
                # via scalar_tensor_tensor's fused accumulator.
                sq = sqp.tile([P, 2048], f32, tag="sq")
                nc.scalar.activation(sq, ps, sqrt_fn, scale=inv_scale)
                for h in range(2):
                    t = 2 * g + h
                    base = h * 1024
                    dummy = sqp.tile([P, 512], f32, tag="dm")
                    nc.vector.scalar_tensor_tensor(
                        out=dummy, in0=sq[:, base:base + 512], scalar=0.0,
                        in1=sq[:, base + 512:base + 1024],
                        op0=add, op1=add, accum_out=wd[:, t:t + 1])

            out_sb = accp.tile([P, TILES], f32, tag="os")
            nc.vector.tensor_scalar(out=out_sb, in0=wd,
                                    scalar1=-1.0, scalar2=mad_sb,
                                    op0=mybir.AluOpType.mult,
                                    op1=mybir.AluOpType.add)
            nc.sync.dma_start(out=out[:, :], in_=out_sb)
    nc.finalize()
    return nc


def _get_nc():
    global _nc_cache
    if _nc_cache is None:
        _nc_cache = _build_nc()
    return _nc_cache


def build_in_maps(inputs, centers, coefs, max_avg_distance):
    x = np.ascontiguousarray(np.asarray(inputs, dtype=np.float32).reshape(N, D))
    cen = np.asarray(centers, dtype=np.float32)
    co = np.asarray(coefs, dtype=np.float32)
    mad = np.asarray(max_avg_distance, dtype=np.float32).reshape(1)

    w = np.abs(co)
    s = np.float32(w.sum(dtype=np.float32))
    if s != 0.0:
        w = (w / s).astype(np.float32)
    u = (w.astype(np.float64) ** 2) * (2.0 ** S)
    c2 = (cen.astype(np.float64) ** 2).sum(axis=1)

    cw = np.empty((K, C), dtype=np.float16)
    cw[:D] = (-2.0 * u[:, None] * cen.astype(np.float64)).T.astype(np.float16)
    cw[D] = u.astype(np.float16)
    cw[D + 1] = cw[D]
    cw[D + 2] = (u * c2).astype(np.float16)

    in_maps = []
    for g in range(N_CORES):
        xg = x[g * NS:(g + 1) * NS]
        x2 = (xg.astype(np.float64) ** 2).sum(axis=1)
        x2_hi = x2.astype(np.float16)
        x2_lo = (x2 - x2_hi.astype(np.float64)).astype(np.float16)
        xaT = np.empty((K, NS), dtype=np.float16)
        xaT[:D] = xg.T.astype(np.float16)
        xaT[D] = x2_hi
        xaT[D + 1] = x2_lo
        xaT[D + 2] = 1.0
        in_maps.append({"xaT": xaT, "cw": cw, "mad": mad})
    return in_maps


def kernel(inputs, centers, coefs, max_avg_distance):
    in_maps = build_in_maps(inputs, centers, coefs, max_avg_distance)
    res = run_bass_kernel_spmd(_get_nc(), in_maps, core_ids=list(range(N_CORES)))
    full = np.concatenate(
        [np.asarray(res.results[g]["out"]).T.reshape(-1) for g in range(N_CORES)]
    )
    return full.astype(np.float32)
